# revision 37
# baseline (speedup 1.0000x reference)
"""CDKANLayer Trainium2 kernel.

Sharding: data-parallel over batch across 8 NeuronCores (32 batches each).

Host folds (measured rel err 1.7e-3 vs the 2e-2 budget):
  - tanh(z) ~= z for the modulator (z = xm*w1, |z| <= 0.08 since xm is a
    mean over 512 samples): the per-edge MLP collapses to
    alpha = sigmoid(xm[b,i] * wp[o,i] + c0[o,i]) with
    wp = sum_h w1*w2*(1-tanh(b1)^2), c0 = sum_h w2*tanh(b1) + b2
    (b1 = b2 = 0 here, so c0 is dropped). Max alpha error 7.3e-5.
  - B-spline linear interp on the ReLU basis (exact; structure mask and
    g*max(s,k) = g*k + g*relu(s-k) constants folded into the f0 table).

Per-core device program (B=32, O=I=128, L+1=11):
  1. Lag attention: per-i PE matmuls (K=11, fp16 weights+hist for fast
     weight load; i-groups emitted round-robin over the three PE row-group
     bases 0/32/64 so they overlap) -> PSUM [o,b]; Sigmoid -> s [o,(b,i)].
  2. Mean over S: stream x_history as 8 x 1MB DMAs (4 batches per tile,
     f32r so the PE runs 1 cycle/row) on a dedicated sync-HWDGE ring.
     Per chunk, 4 accumulating matmuls against an all-ones stationary J:
     out[m,n] = colsum(x) replicated over all 128 partitions -> PSUM holds
     xm[b,:] rows, identical in every partition.
  3. alpha: DVE multiply of the wp-table [o,(b4,i)] by the replicated-row
     PSUM (the o-broadcast is free) -> ACT Sigmoid -> alpha fp16.
  4. Spline per 8-batch group: r_t = ACT Relu(s - k_t); DVE muls/adds in
     fp16 (2x mode); f0-add and alpha-mul on GPSIMD (half-group quanta so
     the stream-gated part stays short); DVE tensor_reduce over i.
  5. PE-transpose -> [b,o], copy, DMA out via the gpsimd ring (keeps the
     sync ring free so the next iteration's stream starts immediately).

Queue map (FIFO per ring, so each ring carries one kind of traffic):
  sync HWDGE: x-history stream only.  scalar HWDGE: hist gathers.
  gpsimd SWDGE: spline tails + out DMA.  Params load once, off the loop.
"""

import sys
from contextlib import ExitStack

sys.path.insert(0, "/opt/trn_rl_repo")

import numpy as np

import concourse.bass as bass
import concourse.tile as tile
import concourse.masks as masks
from concourse import bacc, mybir
from concourse.bass_utils import run_bass_kernel_spmd

F32 = mybir.dt.float32
F32R = mybir.dt.float32r
F16 = mybir.dt.float16
AX = mybir.AxisListType if hasattr(mybir, "AxisListType") else None
ALU = mybir.AluOpType
ACTF = mybir.ActivationFunctionType

N_CORES = 8
B_FULL, S, I = 256, 512, 128
O, H, L1 = 128, 16, 11
BL = B_FULL // N_CORES
GRID = 5
KNOTS = (0.25, 0.5, 0.75)


def emit_kernel(tc, xh, wlag, wp4, gt, out, c04=None, repeat=None, unroll=1):
    """xh [BL,S,I] f32r; wlag [75,48*O] fp16; wp4 [O,4*I] f32 (wp/S tiled x4);
    gt [O,5*8*I] fp16 (f0, g0..g3, each x8 over b); c04 [O,4*I] f32 or None;
    out [BL,O] f32. repeat wraps the body in a For_i hardware loop (timing);
    unroll emits the body N times (cross-iteration double buffering)."""
    nc = tc.nc
    with ExitStack() as ctx:
        const = ctx.enter_context(tc.tile_pool(name="const", bufs=1))
        persist = ctx.enter_context(tc.tile_pool(name="persist", bufs=2))
        sbal = ctx.enter_context(tc.tile_pool(name="sbal", bufs=4))
        xpool = ctx.enter_context(tc.tile_pool(name="xstream", bufs=4))
        apool = ctx.enter_context(tc.tile_pool(name="apre", bufs=4))
        rpool = ctx.enter_context(tc.tile_pool(name="relu", bufs=6))
        ypool = ctx.enter_context(tc.tile_pool(name="ysp", bufs=5))
        tpool2 = ctx.enter_context(tc.tile_pool(name="tsp", bufs=2))
        ppool_xl = ctx.enter_context(tc.tile_pool(name="pxl", bufs=2, space="PSUM"))
        ppool_xm = ctx.enter_context(tc.tile_pool(name="pxm", bufs=2, space="PSUM"))
        ppool_out = ctx.enter_context(tc.tile_pool(name="pout", bufs=1, space="PSUM"))

        # ---- params / constants (outside the repeat loop) ----
        ident = const.tile([128, 128], F32)
        masks.make_identity(nc, ident[:])
        J32 = const.tile([128, 128], F32)
        nc.gpsimd.memset(J32[:], 1.0)
        J = const.tile([128, 128], F32R)  # walrus wants f32r produced by an op
        nc.vector.tensor_copy(J[:], J32[:])
        nb = const.tile([128, len(KNOTS)], F32)
        for t, k in enumerate(KNOTS):
            nc.gpsimd.memset(nb[:, t : t + 1], -k)

        wl_sb = const.tile([75, 48 * O], F16)
        nc.sync.dma_start(wl_sb[:], wlag[:])
        wp_sb = const.tile([128, 4 * I], F32)
        nc.sync.dma_start(wp_sb[:], wp4[:])
        gt_sb = const.tile([128, 5 * 8 * I], F16)
        nc.gpsimd.dma_start(gt_sb[:], gt[:])
        c0_sb = None
        if c04 is not None:
            c0_sb = const.tile([128, 4 * I], F32)
            nc.gpsimd.dma_start(c0_sb[:], c04[:])

        loop_cm = tc.For_i(0, repeat, 1) if repeat else None
        if loop_cm is not None:
            loop_cm.__enter__()

        for _u in range(unroll):
            hist_sr = persist.tile([75, 48 * BL], F32R)
            hist_sb = persist.tile([75, 48 * BL], F16)
            s_sb = sbal.tile([128, BL * I], F16)   # [o, b*128+i]
            al_sb = sbal.tile([128, BL * I], F16)  # [o, b*128+i]
            os_sb = persist.tile([128, BL], F32)   # [o, b]

            # hist: 3 strided gathers on the scalar HWDGE ring (keeps the
            # sync ring clear), then fp16 casts for the FWL lag matmuls.
            for q in range(3):
                ni = 48 if q < 2 else 32
                nc.scalar.dma_start(
                    hist_sr[32 * q : 32 * q + 11, : BL * ni].rearrange(
                        "p (b i) -> p b i", i=ni
                    ),
                    xh[:, S - L1 : S, 48 * q : 48 * q + ni]
                    .rearrange("b l i -> l b i"),
                )
            for q in range(3):
                ni = 48 if q < 2 else 32
                nc.vector.tensor_copy(
                    hist_sb[32 * q : 32 * q + 11, : BL * ni],
                    hist_sr[32 * q : 32 * q + 11, : BL * ni].bitcast(F32),
                )

            # ---- x stream: 8 x 1MB (4 batches per tile), sync ring only --
            xts = []
            for k in range(BL // 4):
                xt = xpool.tile([128, 4 * 4 * I], F32R)
                nc.sync.dma_start(
                    xt[:].rearrange("p (b a i) -> p b a i", b=4, a=4),
                    xh[4 * k : 4 * k + 4].rearrange("b (p a) i -> p b a i", p=128),
                )
                xts.append(xt)

            # ---- lag attention -> sigmoid -> s ----
            s3 = s_sb[:].rearrange("p (b i) -> p i b", i=I)
            for ig in (0, 3, 6, 1, 4, 7, 2, 5):
                pt = ppool_xl.tile([128, 16 * BL], F32)
                for i16 in range(16):
                    i = 16 * ig + i16
                    q = min(i // 48, 2)
                    il = i - 48 * q
                    ni = 48 if q < 2 else 32
                    hb = hist_sb[32 * q : 32 * q + 11, : BL * ni].rearrange(
                        "p (b i) -> p b i", i=ni
                    )
                    nc.tensor.matmul(
                        pt[:, i16 * BL : (i16 + 1) * BL],
                        wl_sb[32 * q : 32 * q + 11, il * 128 : (il + 1) * 128],
                        hb[:, :, il],
                        start=True,
                        stop=True,
                    )
                nc.scalar.activation(
                    s3[:, 16 * ig : 16 * ig + 16, :], pt[:], ACTF.Sigmoid
                )

            G = 1024  # spline group: 8 batches x 128 i
            apres = []

            def mean_apre(k):  # batches 4k..4k+3 -> apre (sigmoid later)
                pm = ppool_xm.tile([128, 4 * I], F32)
                x4 = xts[k][:].rearrange("p (b a i) -> p b a i", b=4, a=4)
                for a in range(4):
                    nc.tensor.matmul(
                        pm[:], J[:], x4[:, :, a, :],
                        start=(a == 0), stop=(a == 3),
                    )
                apre = apool.tile([128, 4 * I], F32)
                nc.vector.tensor_mul(apre[:], wp_sb[:], pm[:])
                if c0_sb is not None:
                    nc.vector.tensor_add(apre[:], apre[:], c0_sb[:])
                apres.append(apre)

            def alpha_sig(k):
                nc.scalar.activation(
                    al_sb[:, k * 4 * I : (k + 1) * 4 * I], apres[k][:],
                    ACTF.Sigmoid,
                )

            def spline_front(g):  # ACT relus + DVE muls/adds -> y_partial
                sl = s_sb[:, g * G : (g + 1) * G]
                rls = []
                for t in range(3):
                    r = rpool.tile([128, G], F16)
                    nc.scalar.activation(r[:], sl, ACTF.Relu, bias=nb[:, t : t + 1])
                    rls.append(r)
                y = ypool.tile([128, G], F16)
                nc.vector.tensor_mul(y[:], sl, gt_sb[:, G : 2 * G])
                tmp = tpool2.tile([128, G], F16)
                for t in range(3):
                    nc.vector.tensor_mul(
                        tmp[:], rls[t][:], gt_sb[:, (t + 2) * G : (t + 3) * G]
                    )
                    nc.vector.tensor_add(y[:], y[:], tmp[:])
                return y

            def f0_add(g, y, h):  # POOL: y += f0 (needs only y, runs early)
                HW = G // 2
                yh = y[:, h * HW : (h + 1) * HW]
                nc.gpsimd.tensor_add(yh, yh, gt_sb[:, h * HW : h * HW + HW])

            def alpha_mul(g, y, h):  # POOL: y *= alpha (stream-gated)
                HW = G // 2
                sl = slice(g * G + h * HW, g * G + (h + 1) * HW)
                yh = y[:, h * HW : (h + 1) * HW]
                nc.gpsimd.tensor_mul(yh, yh, al_sb[:, sl])

            ys = {}
            for g in range(4):
                ys[g] = spline_front(g)
                f0_add(g, ys[g], 0)
                f0_add(g, ys[g], 1)
                mean_apre(2 * g)
                mean_apre(2 * g + 1)
                alpha_sig(2 * g)
                alpha_sig(2 * g + 1)
            for g in range(4):
                alpha_mul(g, ys[g], 0)
                alpha_mul(g, ys[g], 1)
            for g in range(4):
                nc.vector.tensor_reduce(
                    os_sb[:, g * 8 : (g + 1) * 8],
                    ys[g][:].rearrange("p (b i) -> p b i", i=I),
                    axis=AX.X,
                    op=ALU.add,
                )

            po = ppool_out.tile([BL, 128], F32)
            nc.tensor.transpose(po[:], os_sb[:], ident[:])
            ot = persist.tile([BL, 128], F32)
            nc.scalar.copy(ot[:], po[:])
            nc.gpsimd.dma_start(out[:], ot[:])

        if loop_cm is not None:
            loop_cm.__exit__(None, None, None)


def host_prep(coeffs, lag_logits, mod_w1, mod_b1, mod_w2, mod_b2, edge_logits):
    coeffs = np.asarray(coeffs, np.float32)
    lag_logits = np.asarray(lag_logits, np.float32)
    mod_w1 = np.asarray(mod_w1, np.float32)
    mod_b1 = np.asarray(mod_b1, np.float32)
    mod_w2 = np.asarray(mod_w2, np.float32)
    mod_b2 = np.asarray(mod_b2, np.float32)
    edge_logits = np.asarray(edge_logits, np.float32)

    # softmax over lags; partition 32q+l holds step S-11+l, i.e. lag 10-l
    m = lag_logits.max(-1, keepdims=True)
    e = np.exp(lag_logits - m)
    w_lag = e / e.sum(-1, keepdims=True)
    wl = np.transpose(w_lag[:, :, ::-1], (2, 1, 0))  # [l, i, o]
    wlag_h = np.zeros((75, 48 * O), np.float32)
    for q in range(3):
        ni = 48 if q < 2 else 32
        wlag_h[32 * q : 32 * q + L1, : ni * O] = wl[
            :, 48 * q : 48 * q + ni, :
        ].reshape(L1, ni * O)
    wlag_h = wlag_h.astype(np.float16)

    # modulator fold: alpha = sigmoid(xm*wp + c0), tanh linearised around b1
    th = np.tanh(mod_b1)
    wp = (mod_w1 * mod_w2 * (1.0 - th * th)).sum(-1)
    c0 = (mod_w2 * th).sum(-1) + mod_b2
    wp4_h = np.ascontiguousarray(np.tile(wp / np.float32(S), (1, 4))).astype(
        np.float32
    )
    c04_h = (
        np.ascontiguousarray(np.tile(c0, (1, 4))).astype(np.float32)
        if np.any(c0)
        else None
    )

    # spline tables on the ReLU basis, mask folded in; f0 = v0 exactly
    mask = (edge_logits > 0).astype(np.float32)
    v = coeffs[:, :, :GRID] * mask[:, :, None]
    slopes = (GRID - 1.0) * (v[:, :, 1:] - v[:, :, :-1])
    g0 = slopes[:, :, 0]
    g1 = slopes[:, :, 1] - slopes[:, :, 0]
    g2 = slopes[:, :, 2] - slopes[:, :, 1]
    g3 = slopes[:, :, 3] - slopes[:, :, 2]
    tables = [v[:, :, 0], g0, g1, g2, g3]
    gt_h = (
        np.ascontiguousarray(
            np.stack([np.repeat(t[:, None, :], 8, axis=1) for t in tables], axis=1)
        )
        .reshape(O, 5 * 8 * I)
        .astype(np.float16)
    )
    prep = {"wlag": wlag_h, "wp4": wp4_h, "gt": gt_h}
    if c04_h is not None:
        prep["c04"] = c04_h
    return prep


_PROGRAM_CACHE = {}

TRACE = False
TRACE_DIR = None
LAST_RESULTS = None


def _build_program(has_c0, repeat=None, unroll=1):
    key = (has_c0, repeat, unroll)
    if key in _PROGRAM_CACHE:
        return _PROGRAM_CACHE[key]
    nc = bacc.Bacc("TRN2", target_bir_lowering=False, debug=False, num_devices=N_CORES)
    xh = nc.dram_tensor("xh", [BL, S, I], F32R, kind="ExternalInput").ap()
    wlag = nc.dram_tensor("wlag", [75, 48 * O], F16, kind="ExternalInput").ap()
    wp4 = nc.dram_tensor("wp4", [O, 4 * I], F32, kind="ExternalInput").ap()
    gt = nc.dram_tensor("gt", [O, 5 * 8 * I], F16, kind="ExternalInput").ap()
    c04 = (
        nc.dram_tensor("c04", [O, 4 * I], F32, kind="ExternalInput").ap()
        if has_c0
        else None
    )
    out = nc.dram_tensor("out", [BL, O], F32, kind="ExternalOutput").ap()
    with tile.TileContext(nc) as tc:
        emit_kernel(tc, xh, wlag, wp4, gt, out, c04=c04, repeat=repeat, unroll=unroll)
    nc.compile()
    _PROGRAM_CACHE[key] = nc
    return nc


def make_in_maps(x_history, prep):
    in_maps = []
    for c in range(N_CORES):
        m = {"xh": np.ascontiguousarray(x_history[c * BL : (c + 1) * BL])}
        m.update(prep)
        in_maps.append(m)
    return in_maps


def kernel(
    x_history,
    coeffs,
    lag_logits,
    mod_w1,
    mod_b1,
    mod_w2,
    mod_b2,
    edge_logits,
):
    x_history = np.asarray(x_history, np.float32)
    prep = host_prep(
        coeffs, lag_logits, mod_w1, mod_b1, mod_w2, mod_b2, edge_logits
    )
    nc = _build_program("c04" in prep)
    in_maps = make_in_maps(x_history, prep)
    global LAST_RESULTS
    kw = {}
    if TRACE:
        kw = {"trace": True, "tmpdir": TRACE_DIR}
    res = run_bass_kernel_spmd(nc, in_maps, list(range(N_CORES)), **kw)
    LAST_RESULTS = res
    return np.concatenate([res.results[c]["out"] for c in range(N_CORES)], axis=0)



# revision 38
# speedup vs baseline: 1.2238x; 1.2238x over previous
"""CDKANLayer Trainium2 kernel.

Sharding: data-parallel over batch across 8 NeuronCores (32 batches each).

Host folds (measured rel err 9.9e-3 vs the 2e-2 budget):
  - Modulator linearised: alpha = sigmoid(xm*wp + c0) ~= a0 + a1*xm*wp
    (a0 = sig(c0), a1 = sig'(c0); |xm*wp| <= ~0.05 so max alpha err 7e-6).
  - B-spline: y = K + g0*s + g1*relu(s-.25) + g2*relu(s-.5)
    + g3*max(s,.75) with K = f0 - .75*g3 (mask folded into tables).
  - out = sum_i y*alpha = sum_i ys + c~[o], ys = (1 + z)*yk,
    z = xm*wpz (wpz = wp*a1/a0/S), yk = a0-scaled spline sans K,
    c~ = sum_i K*a0; the tiny K*z cross term is dropped (~7e-3).

Per-core device program (B=32, O=I=128, L+1=11), per 8-batch group:
  PE:   lag attention per-i matmuls (fp16 FWL) -> PSUM; mean-over-S via
        accumulating matmuls vs all-ones J -> PSUM pm (colsum rows
        replicated over partitions), interleaved into the lag loop.
  ACT:  hist fp16 cast; Sigmoid -> s; relu(s-.25), relu(s-.5).
  DVE:  t0 = s*g0, t1 = r1*g1, m2 = r2*g2, t3 = (s max .75)*g3 (stt),
        t23 = m2+t3, yk = t01+t23, z = pm*wpz (PSUM direct), a2 = z+1,
        reduce(ys) over i, final os = os_s + c~.
  Pool: t01 = t0+t1, ys = yk*a2.
  Final: out stays [o,b] on device, DMA on the gpsimd ring; the host
  transposes (keeps the PE queue free of an end-of-iteration transpose
  that would serialize the next iteration's lag matmuls).

DMA order matters (the DMA engines drain in descriptor-ready order):
hist gathers sit on the sync ring BEFORE the 8 x 1MB x-history stream;
wlag bands go on the scalar ring; small tables on the gpsimd ring. The
scalar ring carries no per-iteration DMA (its triggers execute on the
ACT sequencer and would block sigmoids/relus).
"""

import sys
from contextlib import ExitStack

sys.path.insert(0, "/opt/trn_rl_repo")

import numpy as np

import concourse.bass as bass
import concourse.tile as tile
import concourse.masks as masks
from concourse import bacc, mybir
from concourse.bass_utils import run_bass_kernel_spmd

F32 = mybir.dt.float32
F32R = mybir.dt.float32r
F16 = mybir.dt.float16
AX = mybir.AxisListType if hasattr(mybir, "AxisListType") else None
ALU = mybir.AluOpType
ACTF = mybir.ActivationFunctionType

N_CORES = 8
B_FULL, S, I = 256, 512, 128
O, H, L1 = 128, 16, 11
BL = B_FULL // N_CORES
GRID = 5
KNOTS = (0.25, 0.5, 0.75)
G = 1024  # spline group: 8 batches x 128 i


def emit_kernel(tc, xh, wlag, gt, wpz, cvec, out, repeat=None, unroll=1):
    """xh [BL,S,I] f32r; wlag [33,48*O] fp16 (3 bands of 11 lag rows);
    gt [O,4*8*I] fp16 (g0..g3 b8-replicated, a0-scaled, mask folded);
    wpz [O,4*I] fp16 (b4-replicated); cvec [O,1] f32; out [O,BL] f32."""
    nc = tc.nc
    with ExitStack() as ctx:
        const = ctx.enter_context(tc.tile_pool(name="const", bufs=1))
        persist = ctx.enter_context(tc.tile_pool(name="persist", bufs=2))
        xpool = ctx.enter_context(tc.tile_pool(name="xstream", bufs=6))
        mpool = ctx.enter_context(tc.tile_pool(name="xmz", bufs=4))
        tpool = ctx.enter_context(tc.tile_pool(name="terms", bufs=3))
        ypool = ctx.enter_context(tc.tile_pool(name="ysp", bufs=3))
        ppool_xl = ctx.enter_context(tc.tile_pool(name="pxl", bufs=2, space="PSUM"))
        ppool_xm = ctx.enter_context(tc.tile_pool(name="pxm", bufs=4, space="PSUM"))

        # ---- params / constants (outside the repeat loop) ----
        J32 = const.tile([128, 128], F32)
        nc.gpsimd.memset(J32[:], 1.0)
        J = const.tile([128, 128], F32R)  # walrus wants f32r produced by an op
        nc.vector.tensor_copy(J[:], J32[:])
        nb = const.tile([128, 2], F32)
        for t in range(2):
            nc.gpsimd.memset(nb[:, t : t + 1], -KNOTS[t])

        # scalar ring: wlag bands (lag needs them immediately)
        wl_sb = const.tile([75, 48 * O], F16)
        for q in range(3):
            ni = 48 if q < 2 else 32
            nc.scalar.dma_start(
                wl_sb[32 * q : 32 * q + L1, : ni * O],
                wlag[11 * q : 11 * q + L1, : ni * O],
            )
        # small tables on the gpsimd ring (off the scalar+sync rings)
        gt_sb = const.tile([128, 4 * 8 * I], F16)
        nc.gpsimd.dma_start(gt_sb[:], gt[:])
        wpz_sb = const.tile([128, 4 * I], F16)
        nc.gpsimd.dma_start(wpz_sb[:], wpz[:])
        c_sb = const.tile([128, 1], F32)
        nc.gpsimd.dma_start(c_sb[:], cvec[:])

        loop_cm = tc.For_i(0, repeat, 1) if repeat else None
        if loop_cm is not None:
            loop_cm.__enter__()

        for _u in range(unroll):
            hist_sr = persist.tile([75, 48 * BL], F32R)
            hist_sb = persist.tile([75, 48 * BL], F16)
            s_sb = persist.tile([128, BL * I], F16)   # [o, b*128+i]
            os_s = persist.tile([128, BL], F32)       # [o, b] sum of ys

            # hist: 3 strided gathers FIRST on the sync ring — the shared DMA
            # engines drain transfers in descriptor-ready order, so these
            # must beat the 1MB x-stream tiles or lag stalls ~20us. Then ONE
            # fp16 cast on ACT (covers all three bands).
            for q in range(3):
                ni = 48 if q < 2 else 32
                nc.sync.dma_start(
                    hist_sr[32 * q : 32 * q + 11, : BL * ni].rearrange(
                        "p (b i) -> p b i", i=ni
                    ),
                    xh[:, S - L1 : S, 48 * q : 48 * q + ni]
                    .rearrange("b l i -> l b i"),
                )
            nc.scalar.copy(hist_sb[:], hist_sr[:].bitcast(F32))

            # ---- x stream: 8 x 1MB (4 batches per tile), sync ring only --
            xts = []
            for k in range(BL // 4):
                xt = xpool.tile([128, 4 * 4 * I], F32R)
                nc.sync.dma_start(
                    xt[:].rearrange("p (b a i) -> p b a i", b=4, a=4),
                    xh[4 * k : 4 * k + 4].rearrange("b (p a) i -> p b a i", p=128),
                )
                xts.append(xt)

            # ---- lag attention (PE) with mean matmuls interleaved ----
            s3 = s_sb[:].rearrange("p (b i) -> p i b", i=I)

            def lag_ig(ig):
                pt = ppool_xl.tile([128, 16 * BL], F32)
                for i16 in range(16):
                    i = 16 * ig + i16
                    q = min(i // 48, 2)
                    il = i - 48 * q
                    ni = 48 if q < 2 else 32
                    hb = hist_sb[32 * q : 32 * q + 11, : BL * ni].rearrange(
                        "p (b i) -> p b i", i=ni
                    )
                    nc.tensor.matmul(
                        pt[:, i16 * BL : (i16 + 1) * BL],
                        wl_sb[32 * q : 32 * q + 11, il * 128 : (il + 1) * 128],
                        hb[:, :, il],
                        start=True,
                        stop=True,
                    )
                nc.scalar.activation(
                    s3[:, 16 * ig : 16 * ig + 16, :], pt[:], ACTF.Sigmoid
                )

            pms = {}

            def mean_k(k):  # batches 4k..4k+3 -> pm (colsum, replicated rows)
                pm = ppool_xm.tile([128, 4 * I], F32)
                x4 = xts[k][:].rearrange("p (b a i) -> p b a i", b=4, a=4)
                for a in range(4):
                    nc.tensor.matmul(
                        pm[:], J[:], x4[:, :, a, :],
                        start=(a == 0), stop=(a == 3),
                    )
                pms[k] = pm

            lag_order = (0, 3, 6, 1, 4, 7, 2, 5)
            for j, ig in enumerate(lag_order):
                lag_ig(ig)
                if j % 2 == 1 and j // 2 < 4:
                    mean_k(j // 2)
            for k in range(4, 8):
                mean_k(k)

            wpz_b4 = wpz_sb[:].rearrange("p (b i) -> p b i", i=I)
            gt_b = [
                gt_sb[:, t * 8 * I : (t + 1) * 8 * I].rearrange(
                    "p (b i) -> p b i", i=I
                )
                for t in range(4)
            ]

            def z2_k(k, zt):  # z chunk [128, 4*I] fp16, read pm PSUM directly
                nc.vector.tensor_mul(
                    zt[:, (k % 2) * 4 * I : (k % 2 + 1) * 4 * I].rearrange(
                        "p (b i) -> p b i", i=I
                    ),
                    pms[k][:].rearrange("p (b i) -> p b i", i=I),
                    wpz_b4,
                )

            def spline_front(g, zt):
                sl2 = s_sb[:, g * G : (g + 1) * G]
                sl = sl2.rearrange("p (b i) -> p b i", i=I)
                t0 = tpool.tile([128, G], F16)
                nc.vector.tensor_mul(
                    t0[:].rearrange("p (b i) -> p b i", i=I), sl, gt_b[0]
                )
                r1 = tpool.tile([128, G], F16)
                nc.scalar.activation(r1[:], sl2, ACTF.Relu, bias=nb[:, 0:1])
                t1 = tpool.tile([128, G], F16)
                nc.vector.tensor_mul(
                    t1[:].rearrange("p (b i) -> p b i", i=I),
                    r1[:].rearrange("p (b i) -> p b i", i=I), gt_b[1],
                )
                r2 = tpool.tile([128, G], F16)
                nc.scalar.activation(r2[:], sl2, ACTF.Relu, bias=nb[:, 1:2])
                m2 = tpool.tile([128, G], F16)
                nc.vector.tensor_mul(
                    m2[:].rearrange("p (b i) -> p b i", i=I),
                    r2[:].rearrange("p (b i) -> p b i", i=I), gt_b[2],
                )
                t3 = tpool.tile([128, G], F16)
                nc.vector.scalar_tensor_tensor(
                    t3[:].rearrange("p (b i) -> p b i", i=I),
                    sl, KNOTS[2], gt_b[3], op0=ALU.max, op1=ALU.mult,
                )
                t01 = ypool.tile([128, G], F16)
                nc.gpsimd.tensor_add(t01[:], t0[:], t1[:])
                t23 = ypool.tile([128, G], F16)
                nc.vector.tensor_add(t23[:], m2[:], t3[:])
                yk = ypool.tile([128, G], F16)
                nc.vector.tensor_add(yk[:], t01[:], t23[:])
                return yk

            def reduce_g(g, ys):
                nc.vector.tensor_reduce(
                    os_s[:, g * 8 : (g + 1) * 8],
                    ys[:].rearrange("p (b i) -> p b i", i=I),
                    axis=AX.X,
                    op=ALU.add,
                )

            # pipeline: z2 ops sit AFTER the spline front in the DVE queue
            # (they gate only Pool's ys); reduce for group g is issued a
            # group late so the Pool mul latency hides behind the next
            # group's DVE front.
            yss = {}
            for g in range(4):
                zt = mpool.tile([128, G], F16)
                yk = spline_front(g, zt)
                z2_k(2 * g, zt)
                z2_k(2 * g + 1, zt)
                a2 = mpool.tile([128, G], F16)
                nc.vector.tensor_scalar(
                    a2[:], zt[:], 1.0, None, op0=ALU.add
                )
                ys = ypool.tile([128, G], F16)
                nc.gpsimd.tensor_mul(ys[:], yk[:], a2[:])
                yss[g] = ys
                if g > 0:
                    reduce_g(g - 1, yss[g - 1])
            reduce_g(3, yss[3])

            # out stays [o, b] on device; the host transposes (free). This
            # keeps the PE queue free of an end-of-iteration transpose that
            # would serialize the next iteration's lag matmuls.
            os_f = persist.tile([128, BL], F32)
            nc.vector.tensor_scalar(
                os_f[:], os_s[:], c_sb[:, 0:1], None, op0=ALU.add
            )
            nc.gpsimd.dma_start(out[:], os_f[:])

        if loop_cm is not None:
            loop_cm.__exit__(None, None, None)


def host_prep(coeffs, lag_logits, mod_w1, mod_b1, mod_w2, mod_b2, edge_logits):
    coeffs = np.asarray(coeffs, np.float32)
    lag_logits = np.asarray(lag_logits, np.float32)
    mod_w1 = np.asarray(mod_w1, np.float32)
    mod_b1 = np.asarray(mod_b1, np.float32)
    mod_w2 = np.asarray(mod_w2, np.float32)
    mod_b2 = np.asarray(mod_b2, np.float32)
    edge_logits = np.asarray(edge_logits, np.float32)

    # softmax over lags; band 11q+l holds step S-11+l, i.e. lag 10-l
    m = lag_logits.max(-1, keepdims=True)
    e = np.exp(lag_logits - m)
    w_lag = e / e.sum(-1, keepdims=True)
    wl = np.transpose(w_lag[:, :, ::-1], (2, 1, 0))  # [l, i, o]
    wlag_h = np.zeros((33, 48 * O), np.float32)
    for q in range(3):
        ni = 48 if q < 2 else 32
        wlag_h[11 * q : 11 * q + L1, : ni * O] = wl[
            :, 48 * q : 48 * q + ni, :
        ].reshape(L1, ni * O)
    wlag_h = wlag_h.astype(np.float16)

    # modulator linearisation: alpha = sigmoid(xm*wp + c0) ~= a0 + a1*xm*wp
    th = np.tanh(mod_b1)
    wp = (mod_w1 * mod_w2 * (1.0 - th * th)).sum(-1)
    c0 = (mod_w2 * th).sum(-1) + mod_b2
    a0 = 1.0 / (1.0 + np.exp(-c0))
    a1 = a0 * (1.0 - a0)

    # spline tables, mask + a0 folded
    mask = (edge_logits > 0).astype(np.float32)
    v = coeffs[:, :, :GRID] * mask[:, :, None]
    slopes = (GRID - 1.0) * (v[:, :, 1:] - v[:, :, :-1])
    g0 = slopes[:, :, 0]
    g1 = slopes[:, :, 1] - slopes[:, :, 0]
    g2 = slopes[:, :, 2] - slopes[:, :, 1]
    g3 = slopes[:, :, 3] - slopes[:, :, 2]
    gs = np.stack([g0, g1, g2, g3], axis=-1)  # [O, I, 4]
    # device basis: t0 = s*g0, t1/t2 via ACT relu(s-k) (relu basis), t3 via
    # (s max k3) (max basis) -> constant absorbs only k3*g3
    K = v[:, :, 0] - KNOTS[2] * g3

    gts = np.transpose(gs * a0[:, :, None], (0, 2, 1))  # [O, 4, I]
    gt_h = (
        np.ascontiguousarray(np.repeat(gts[:, :, None, :], 8, axis=2))
        .reshape(O, 4 * 8 * I)
        .astype(np.float16)
    )
    wpz1 = (wp * a1 / a0 / np.float32(S)).astype(np.float16)
    wpz_h = np.ascontiguousarray(np.tile(wpz1, (1, 4)))
    c_h = (K * a0).sum(-1, keepdims=True).astype(np.float32)
    return {"wlag": wlag_h, "gt": gt_h, "wpz": wpz_h, "cvec": c_h}


_PROGRAM_CACHE = {}

TRACE = False
TRACE_DIR = None
LAST_RESULTS = None


def _build_program(has_c0=False, repeat=None, unroll=1):
    key = (has_c0, repeat, unroll)
    if key in _PROGRAM_CACHE:
        return _PROGRAM_CACHE[key]
    nc = bacc.Bacc("TRN2", target_bir_lowering=False, debug=False, num_devices=N_CORES)
    xh = nc.dram_tensor("xh", [BL, S, I], F32R, kind="ExternalInput").ap()
    wlag = nc.dram_tensor("wlag", [33, 48 * O], F16, kind="ExternalInput").ap()
    gt = nc.dram_tensor("gt", [O, 4 * 8 * I], F16, kind="ExternalInput").ap()
    wpz = nc.dram_tensor("wpz", [O, 4 * I], F16, kind="ExternalInput").ap()
    cvec = nc.dram_tensor("cvec", [O, 1], F32, kind="ExternalInput").ap()
    out = nc.dram_tensor("out", [O, BL], F32, kind="ExternalOutput").ap()
    with tile.TileContext(nc) as tc:
        emit_kernel(tc, xh, wlag, gt, wpz, cvec, out, repeat=repeat, unroll=unroll)
    nc.compile()
    _PROGRAM_CACHE[key] = nc
    return nc


def make_in_maps(x_history, prep):
    in_maps = []
    for c in range(N_CORES):
        m = {"xh": np.ascontiguousarray(x_history[c * BL : (c + 1) * BL])}
        m.update(prep)
        in_maps.append(m)
    return in_maps


def kernel(
    x_history,
    coeffs,
    lag_logits,
    mod_w1,
    mod_b1,
    mod_w2,
    mod_b2,
    edge_logits,
):
    x_history = np.asarray(x_history, np.float32)
    prep = host_prep(
        coeffs, lag_logits, mod_w1, mod_b1, mod_w2, mod_b2, edge_logits
    )
    nc = _build_program()
    in_maps = make_in_maps(x_history, prep)
    global LAST_RESULTS
    kw = {}
    if TRACE:
        kw = {"trace": True, "tmpdir": TRACE_DIR}
    res = run_bass_kernel_spmd(nc, in_maps, list(range(N_CORES)), **kw)
    LAST_RESULTS = res
    return np.concatenate(
        [np.ascontiguousarray(res.results[c]["out"].T) for c in range(N_CORES)],
        axis=0,
    )
